# revision 4
# baseline (speedup 1.0000x reference)
"""MoE CouncilLayer kernel for 8x TRN2 NeuronCores (expert-parallel).

Problem (all-expert MoE, B=2, T=1024, C=768, E=32, H=3072):
    gates = softmax(x @ gate_w + gate_b)                     # [N, E]
    h     = gelu(einsum('nc,ech->neh', x, w1) + b1)          # [N, E, H]
    y     = einsum('neh,ehc->nec', h, w2) + b2               # [N, E, C]
    out   = einsum('ne,nec->nc', gates, y)                   # [N, C]

Sharding: expert-parallel, 4 experts per core; x replicated. Each core
computes its 4 experts' gate-weighted partial sum; host adds the 8
partials (the all-reduce is a cheap 6 MB/core host-side sum).

Per-core layout is feature-major (activations stored [feature, token]):
    mm1: psum[h_blk, t] += w1[c_blk, h_blk].T @ xT[c_blk, t]   (fp16)
    hg  = gelu(psum + b1) * gate_bcast                         (fp16)
    mm2: psum[c_blk, t] += w2[h_blk, c_blk].T @ hg[h_blk, t]   (fp16)
         (+ rank-4 matmul b2T.T @ g_localT folded into the same psum
          accumulation, so b2's gate-weighted contribution is free)
Gates are computed on-device in fp32 (PE matmuls for logits, ACT exp,
PE one-hot/ones broadcast matmuls over the expert partition axis, DVE
reciprocal), with the gate columns permuted host-side so every core's
4 local experts sit at columns 0..3 (keeps the program core-agnostic).
"""

import numpy as np

import concourse.bass as bass
import concourse.tile as tile
from concourse import bacc, mybir
from concourse.bass_utils import run_bass_kernel_spmd

# Problem dims (hardcoded per harness contract)
B, T, C, E, H = 2, 1024, 768, 32, 3072
N = B * T  # 2048 tokens
NCORES = 8
EL = E // NCORES  # 4 local experts
CB = C // 128  # 6 c-blocks
HB = H // 128  # 24 h-blocks
TCG = 2  # token groups (1024 each)
TG = N // TCG  # 1024
TI = TG // 512  # 512-token chunks per group

F16 = mybir.dt.float16
F32 = mybir.dt.float32
AF = mybir.ActivationFunctionType

_CACHED_NC = None


def build_nc(act=AF.Gelu):
    nc = bacc.Bacc(trn_type="TRN2")

    xT16_d = nc.dram_tensor("xT16", [C, N], F16, kind="ExternalInput")
    xT32_d = nc.dram_tensor("xT32", [C, N], F32, kind="ExternalInput")
    gw_d = nc.dram_tensor("gw", [C, E], F32, kind="ExternalInput")
    gb_d = nc.dram_tensor("gb", [E, 1], F32, kind="ExternalInput")
    sel_d = nc.dram_tensor("sel", [E, EL + 1, 128], F32, kind="ExternalInput")
    w1_d = nc.dram_tensor("w1", [EL, C, H], F16, kind="ExternalInput")
    b1_d = nc.dram_tensor("b1", [128, EL, HB], F32, kind="ExternalInput")
    w2_d = nc.dram_tensor("w2", [EL, H, C], F16, kind="ExternalInput")
    b2T_d = nc.dram_tensor("b2T", [EL, CB, 128], F16, kind="ExternalInput")
    outT_d = nc.dram_tensor("outT", [C, N], F32, kind="ExternalOutput")

    with tile.TileContext(nc) as tc:
        with (
            tc.tile_pool(name="const", bufs=1) as cp,
            tc.tile_pool(name="stream", bufs=1) as sp,
            tc.tile_pool(name="psum", bufs=1, space="PSUM") as pp,
        ):
            # --- resident tiles ---
            xT16_sb = cp.tile([128, CB, N], F16)
            gw_sb = cp.tile([128, CB, E], F32)
            gb_sb = cp.tile([E, 1], F32)
            sel_sb = cp.tile([E, EL + 1, 128], F32)
            b1_sb = cp.tile([128, EL, HB], F32)
            b2T_sb = cp.tile([EL, CB, 128], F16)
            expT_sb = cp.tile([E, N], F32)
            g_bcast_sb = cp.tile([128, EL, N], F16)
            g_localT_sb = cp.tile([EL, N], F16)

            nc.sync.dma_start(
                xT16_sb, xT16_d[:, :].rearrange("(cc p) t -> p cc t", p=128)
            )
            nc.sync.dma_start(gw_sb, gw_d[:, :].rearrange("(cc p) e -> p cc e", p=128))
            nc.sync.dma_start(gb_sb, gb_d[:, :])
            nc.sync.dma_start(sel_sb, sel_d[:, :, :])
            nc.sync.dma_start(b1_sb, b1_d[:, :, :])
            nc.sync.dma_start(b2T_sb, b2T_d[:, :, :])

            # --- gate prologue: expT, g_bcast, g_localT ---
            for t4 in range(N // 512):
                ts = slice(t4 * 512, (t4 + 1) * 512)
                x32_t = sp.tile([128, CB, 512], F32, tag="x32", bufs=2, name="x32t")
                nc.sync.dma_start(
                    x32_t,
                    xT32_d[:, :].rearrange("(cc p) t -> p cc t", p=128)[:, :, ts],
                )
                lg = pp.tile([E, 512], F32, tag="h", bufs=4, name="lg")
                for cc in range(CB):
                    nc.tensor.matmul(
                        lg,
                        gw_sb[:, cc, :],
                        x32_t[:, cc, :],
                        start=(cc == 0),
                        stop=(cc == CB - 1),
                    )
                nc.scalar.activation(expT_sb[:, ts], lg, AF.Exp, bias=gb_sb, scale=1.0)
                dn = pp.tile([128, 512], F32, tag="h", bufs=4, name="dn")
                nc.tensor.matmul(
                    dn, sel_sb[:, EL, :], expT_sb[:, ts], start=True, stop=True
                )
                rc = sp.tile([128, 512], F32, tag="recip", bufs=2, name="rc")
                nc.vector.reciprocal(rc, dn)
                for j in range(EL):
                    nm = pp.tile([128, 512], F32, tag="y", bufs=4, name="nm")
                    nc.tensor.matmul(
                        nm, sel_sb[:, j, :], expT_sb[:, ts], start=True, stop=True
                    )
                    nc.vector.tensor_mul(g_bcast_sb[:, j, ts], nm, rc)
                nc.vector.tensor_mul(g_localT_sb[:, ts], expT_sb[0:EL, ts], rc[0:EL, :])

            # --- main: per token-group, per expert: mm1+gelu+scale, mm2+acc ---
            for tg in range(TCG):
                hg = sp.tile([128, HB, TG], F16, tag="hg", bufs=1, name="hg")
                yac = sp.tile([128, CB, TG], F32, tag="yacc", bufs=1, name="yac")
                for e in range(EL):
                    # mm1: h = gelu(w1.T @ xT + b1) * g
                    for hbg in range(HB // 4):
                        w1t = sp.tile([128, CB, 512], F16, tag="w1", bufs=3, name="w1t")
                        nc.sync.dma_start(
                            w1t,
                            w1_d[e, :, :].rearrange("(cc p) h -> p cc h", p=128)[
                                :, :, hbg * 512 : (hbg + 1) * 512
                            ],
                        )
                        for hbi in range(4):
                            hb = hbg * 4 + hbi
                            hps = [
                                pp.tile([128, 512], F32, tag="h", bufs=4, name="hps")
                                for _ in range(TI)
                            ]
                            for cc in range(CB):
                                for ti in range(TI):
                                    gts = slice(
                                        tg * TG + ti * 512, tg * TG + (ti + 1) * 512
                                    )
                                    nc.tensor.matmul(
                                        hps[ti],
                                        w1t[:, cc, hbi * 128 : (hbi + 1) * 128],
                                        xT16_sb[:, cc, gts],
                                        start=(cc == 0),
                                        stop=(cc == CB - 1),
                                    )
                            for ti in range(TI):
                                gts = slice(
                                    tg * TG + ti * 512, tg * TG + (ti + 1) * 512
                                )
                                lts = slice(ti * 512, (ti + 1) * 512)
                                nc.scalar.activation(
                                    hg[:, hb, lts],
                                    hps[ti],
                                    act,
                                    bias=b1_sb[:, e, hb : hb + 1],
                                    scale=1.0,
                                )
                                nc.vector.tensor_mul(
                                    hg[:, hb, lts],
                                    hg[:, hb, lts],
                                    g_bcast_sb[:, e, gts],
                                )
                    # mm2: y_psum = b2T.T @ g_localT + sum_hb w2.T @ hg
                    for cb in range(CB):
                        w2t = sp.tile([128, HB, 128], F16, tag="w2", bufs=3, name="w2t")
                        nc.sync.dma_start(
                            w2t,
                            w2_d[e, :, :].rearrange("(hb p) c -> p hb c", p=128)[
                                :, :, cb * 128 : (cb + 1) * 128
                            ],
                        )
                        yps = [
                            pp.tile([128, 512], F32, tag="y", bufs=4, name="yps")
                            for _ in range(TI)
                        ]
                        if e == 0:
                            # b2's gate-weighted contribution (summed over all
                            # 4 local experts by the rank-4 matmul), added once
                            for ti in range(TI):
                                gts = slice(
                                    tg * TG + ti * 512, tg * TG + (ti + 1) * 512
                                )
                                nc.tensor.matmul(
                                    yps[ti],
                                    b2T_sb[:, cb, :],
                                    g_localT_sb[:, gts],
                                    start=True,
                                    stop=False,
                                )
                        for hb in range(HB):
                            for ti in range(TI):
                                lts = slice(ti * 512, (ti + 1) * 512)
                                nc.tensor.matmul(
                                    yps[ti],
                                    w2t[:, hb, :],
                                    hg[:, hb, lts],
                                    start=(e != 0 and hb == 0),
                                    stop=(hb == HB - 1),
                                )
                        for ti in range(TI):
                            lts = slice(ti * 512, (ti + 1) * 512)
                            if e == 0:
                                nc.vector.tensor_copy(yac[:, cb, lts], yps[ti])
                            else:
                                nc.vector.tensor_add(
                                    yac[:, cb, lts], yps[ti], yac[:, cb, lts]
                                )
                for cb in range(CB):
                    nc.sync.dma_start(
                        outT_d[cb * 128 : (cb + 1) * 128, tg * TG : (tg + 1) * TG],
                        yac[:, cb, :],
                    )

    nc.compile()
    return nc


def _get_nc():
    global _CACHED_NC
    if _CACHED_NC is None:
        _CACHED_NC = build_nc()
    return _CACHED_NC


def make_in_maps(x, gate_w, gate_b, w1, b1, w2, b2):
    x = np.asarray(x, np.float32)
    gate_w = np.asarray(gate_w, np.float32)
    gate_b = np.asarray(gate_b, np.float32)
    w1 = np.asarray(w1, np.float32)
    b1 = np.asarray(b1, np.float32)
    w2 = np.asarray(w2, np.float32)
    b2 = np.asarray(b2, np.float32)

    xT32 = np.ascontiguousarray(x.reshape(N, C).T)
    xT16 = xT32.astype(np.float16)
    w1_16 = w1.astype(np.float16)
    w2_16 = w2.astype(np.float16)

    sel = np.zeros((E, EL + 1, 128), np.float32)
    for j in range(EL):
        sel[j, j, :] = 1.0
    sel[:, EL, :] = 1.0

    in_maps = []
    for i in range(NCORES):
        lo, hi = EL * i, EL * (i + 1)
        perm = list(range(lo, hi)) + [e for e in range(E) if not (lo <= e < hi)]
        in_maps.append(
            {
                "xT16": xT16,
                "xT32": xT32,
                "gw": np.ascontiguousarray(gate_w[:, perm]),
                "gb": np.ascontiguousarray(gate_b[perm]).reshape(E, 1),
                "sel": sel,
                "w1": w1_16[lo:hi],
                "b1": np.ascontiguousarray(
                    b1[lo:hi].reshape(EL, HB, 128).transpose(2, 0, 1)
                ),
                "w2": w2_16[lo:hi],
                "b2T": b2[lo:hi].reshape(EL, CB, 128).astype(np.float16),
            }
        )
    return in_maps


def kernel(x, gate_w, gate_b, w1, b1, w2, b2, _trace=False, _tmpdir=None):
    nc = _get_nc()
    in_maps = make_in_maps(x, gate_w, gate_b, w1, b1, w2, b2)
    res = run_bass_kernel_spmd(
        nc,
        in_maps,
        core_ids=list(range(NCORES)),
        trace=_trace,
        tmpdir=_tmpdir,
    )
    acc = res.results[0]["outT"].astype(np.float64)
    for r in res.results[1:]:
        acc += r["outT"]
    out = acc.T.reshape(B, T, C).astype(np.float32)
    if _trace:
        kernel._last_results = res
    return out


# revision 8
# speedup vs baseline: 1.0436x; 1.0436x over previous
"""MoE CouncilLayer kernel for 8x TRN2 NeuronCores (expert-parallel).

Problem (all-expert MoE, B=2, T=1024, C=768, E=32, H=3072):
    gates = softmax(x @ gate_w + gate_b)                     # [N, E]
    h     = gelu(einsum('nc,ech->neh', x, w1) + b1)          # [N, E, H]
    y     = einsum('neh,ehc->nec', h, w2) + b2               # [N, E, C]
    out   = einsum('ne,nec->nc', gates, y)                   # [N, C]

Sharding: expert-parallel, 4 experts per core; x replicated. Each core
computes its 4 experts' gate-weighted partial sum; host adds the 8
partials (the all-reduce is a cheap 6 MB/core host-side sum).

Per-core layout is feature-major (activations stored [feature, token]):
    mm1: psum[h_blk, t] += w1[c_blk, h_blk].T @ xT[c_blk, t]   (fp16)
    hg  = gelu(psum + b1) * gate_bcast                         (fp16)
    mm2: psum[c_blk, t] += w2[h_blk, c_blk].T @ hg[h_blk, t]   (fp16)
         (+ rank-4 matmul b2T.T @ g_localT folded into the same psum
          accumulation, so b2's gate-weighted contribution is free)
Gates are computed on-device in fp32 (PE matmuls for logits, ACT exp,
PE one-hot/ones broadcast matmuls over the expert partition axis, DVE
reciprocal), with the gate columns permuted host-side so every core's
4 local experts sit at columns 0..3 (keeps the program core-agnostic).
"""

import numpy as np

import concourse.bass as bass
import concourse.tile as tile
from concourse import bacc, mybir
from concourse.bass_utils import run_bass_kernel_spmd

# Problem dims (hardcoded per harness contract)
B, T, C, E, H = 2, 1024, 768, 32, 3072
N = B * T  # 2048 tokens
NCORES = 8
EL = E // NCORES  # 4 local experts
CB = C // 128  # 6 c-blocks
HB = H // 128  # 24 h-blocks
TCG = 2  # token groups (1024 each)
TG = N // TCG  # 1024
TI = TG // 512  # 512-token chunks per group

F16 = mybir.dt.float16
F32 = mybir.dt.float32
AF = mybir.ActivationFunctionType

_CACHED_NC = None


def build_nc(act=AF.Gelu):
    nc = bacc.Bacc(trn_type="TRN2")

    xT16_d = nc.dram_tensor("xT16", [C, N], F16, kind="ExternalInput")
    gw_d = nc.dram_tensor("gw", [C, E], F16, kind="ExternalInput")
    gb_d = nc.dram_tensor("gb", [E, 1], F32, kind="ExternalInput")
    ones_d = nc.dram_tensor("ones32", [E, EL], F32, kind="ExternalInput")
    w1_d = nc.dram_tensor("w1", [EL, C, H], F16, kind="ExternalInput")
    b1_d = nc.dram_tensor("b1", [128, EL, HB], F32, kind="ExternalInput")
    w2_d = nc.dram_tensor("w2", [EL, H, C], F16, kind="ExternalInput")
    b2T_d = nc.dram_tensor("b2T", [EL, CB, 128], F16, kind="ExternalInput")
    outT_d = nc.dram_tensor("outT", [C, N], F32, kind="ExternalOutput")

    with tile.TileContext(nc) as tc:
        with (
            tc.tile_pool(name="const", bufs=1) as cp,
            tc.tile_pool(name="stream", bufs=1) as sp,
            tc.tile_pool(name="psum", bufs=1, space="PSUM") as pp,
            tc.tile_pool(name="dram", bufs=1, space="DRAM") as dp,
        ):
            # --- resident tiles ---
            xT16_sb = cp.tile([128, CB, N], F16)
            gw_sb = cp.tile([128, CB, E], F16)
            gb_sb = cp.tile([E, 1], F32)
            ones_sb = cp.tile([E, EL], F32)
            b1_sb = cp.tile([128, EL, HB], F32)
            b2T_sb = cp.tile([EL, CB, 128], F16)
            expT_sb = cp.tile([E, N], F32)
            g_bcast_sb = cp.tile([128, EL, N], F16)
            g_localT_sb = cp.tile([EL, N], F16)

            for cc in range(CB):
                nc.sync.dma_start(
                    xT16_sb[:, cc, :],
                    xT16_d[cc * 128 : (cc + 1) * 128, :],
                )
            nc.sync.dma_start(gw_sb, gw_d[:, :].rearrange("(cc p) e -> p cc e", p=128))
            nc.sync.dma_start(gb_sb, gb_d[:, :])
            nc.sync.dma_start(ones_sb, ones_d[:, :])
            nc.sync.dma_start(b1_sb, b1_d[:, :, :])
            nc.sync.dma_start(b2T_sb, b2T_d[:, :, :])

            # --- gate prologue: expT, g_localT; broadcast via DRAM bounce ---
            for t4 in range(N // 512):
                ts = slice(t4 * 512, (t4 + 1) * 512)
                lg = pp.tile([E, 512], F32, tag="h", bufs=4, name="lg")
                for cc in range(CB):
                    nc.tensor.matmul(
                        lg,
                        gw_sb[:, cc, :],
                        xT16_sb[:, cc, ts],
                        start=(cc == 0),
                        stop=(cc == CB - 1),
                    )
                nc.scalar.activation(expT_sb[:, ts], lg, AF.Exp, bias=gb_sb, scale=1.0)
                # denominator (sum over the 32 expert partitions) on PE, fp32
                dn = pp.tile([EL, 512], F32, tag="h", bufs=4, name="dn")
                nc.tensor.matmul(
                    dn, ones_sb[:, :], expT_sb[:, ts], start=True, stop=True
                )
                rc = sp.tile([EL, 512], F32, tag="recip", bufs=2, name="rc")
                nc.vector.reciprocal(rc, dn)
                nc.vector.tensor_mul(g_localT_sb[:, ts], expT_sb[0:EL, ts], rc)
            # bounce local gates through DRAM to broadcast across partitions
            g_dram = dp.tile([EL, N], F16, name="g_dram")
            nc.sync.dma_start(g_dram, g_localT_sb[:, :])
            for j in range(EL):
                nc.sync.dma_start(
                    g_bcast_sb[:, j, :], g_dram[j : j + 1, :].to_broadcast((128, N))
                )

            # --- main: per token-group, per expert: mm1+gelu+scale, mm2+acc ---
            for tg in range(TCG):
                hg = sp.tile([128, HB, TG], F16, tag="hg", bufs=1, name="hg")
                yac = sp.tile([128, CB, TG], F32, tag="yacc", bufs=1, name="yac")
                for e in range(EL):
                    # mm1: h = gelu(w1.T @ xT + b1) * g
                    for hbg in range(HB // 4):
                        w1t = sp.tile([128, CB, 512], F16, tag="w1", bufs=3, name="w1t")
                        nc.sync.dma_start(
                            w1t,
                            w1_d[e, :, :].rearrange("(cc p) h -> p cc h", p=128)[
                                :, :, hbg * 512 : (hbg + 1) * 512
                            ],
                        )
                        for hbi in range(4):
                            hb = hbg * 4 + hbi
                            hps = [
                                pp.tile([128, 512], F32, tag="h", bufs=4, name="hps")
                                for _ in range(TI)
                            ]
                            for cc in range(CB):
                                for ti in range(TI):
                                    gts = slice(
                                        tg * TG + ti * 512, tg * TG + (ti + 1) * 512
                                    )
                                    nc.tensor.matmul(
                                        hps[ti],
                                        w1t[:, cc, hbi * 128 : (hbi + 1) * 128],
                                        xT16_sb[:, cc, gts],
                                        start=(cc == 0),
                                        stop=(cc == CB - 1),
                                    )
                            for ti in range(TI):
                                gts = slice(
                                    tg * TG + ti * 512, tg * TG + (ti + 1) * 512
                                )
                                lts = slice(ti * 512, (ti + 1) * 512)
                                nc.scalar.activation(
                                    hg[:, hb, lts],
                                    hps[ti],
                                    act,
                                    bias=b1_sb[:, e, hb : hb + 1],
                                    scale=1.0,
                                )
                                nc.vector.tensor_mul(
                                    hg[:, hb, lts],
                                    hg[:, hb, lts],
                                    g_bcast_sb[:, e, gts],
                                )
                    # mm2: y_psum = b2T.T @ g_localT + sum_hb w2.T @ hg
                    for cb in range(CB):
                        w2t = sp.tile([128, HB, 128], F16, tag="w2", bufs=3, name="w2t")
                        nc.sync.dma_start(
                            w2t,
                            w2_d[e, :, :].rearrange("(hb p) c -> p hb c", p=128)[
                                :, :, cb * 128 : (cb + 1) * 128
                            ],
                        )
                        yps = [
                            pp.tile([128, 512], F32, tag="y", bufs=4, name="yps")
                            for _ in range(TI)
                        ]
                        if e == 0:
                            # b2's gate-weighted contribution (summed over all
                            # 4 local experts by the rank-4 matmul), added once
                            for ti in range(TI):
                                gts = slice(
                                    tg * TG + ti * 512, tg * TG + (ti + 1) * 512
                                )
                                nc.tensor.matmul(
                                    yps[ti],
                                    b2T_sb[:, cb, :],
                                    g_localT_sb[:, gts],
                                    start=True,
                                    stop=False,
                                )
                        for hb in range(HB):
                            for ti in range(TI):
                                lts = slice(ti * 512, (ti + 1) * 512)
                                nc.tensor.matmul(
                                    yps[ti],
                                    w2t[:, hb, :],
                                    hg[:, hb, lts],
                                    start=(e != 0 and hb == 0),
                                    stop=(hb == HB - 1),
                                )
                        for ti in range(TI):
                            lts = slice(ti * 512, (ti + 1) * 512)
                            if e == 0:
                                nc.vector.tensor_copy(yac[:, cb, lts], yps[ti])
                            else:
                                nc.vector.tensor_add(
                                    yac[:, cb, lts], yps[ti], yac[:, cb, lts]
                                )
                for cb in range(CB):
                    nc.sync.dma_start(
                        outT_d[cb * 128 : (cb + 1) * 128, tg * TG : (tg + 1) * TG],
                        yac[:, cb, :],
                    )

    nc.compile()
    return nc


def _get_nc():
    global _CACHED_NC
    if _CACHED_NC is None:
        _CACHED_NC = build_nc()
    return _CACHED_NC


def make_in_maps(x, gate_w, gate_b, w1, b1, w2, b2):
    x = np.asarray(x, np.float32)
    gate_w = np.asarray(gate_w, np.float32)
    gate_b = np.asarray(gate_b, np.float32)
    w1 = np.asarray(w1, np.float32)
    b1 = np.asarray(b1, np.float32)
    w2 = np.asarray(w2, np.float32)
    b2 = np.asarray(b2, np.float32)

    xT16 = np.ascontiguousarray(x.reshape(N, C).T).astype(np.float16)
    w1_16 = w1.astype(np.float16)
    w2_16 = w2.astype(np.float16)

    ones32 = np.ones((E, EL), np.float32)

    in_maps = []
    for i in range(NCORES):
        lo, hi = EL * i, EL * (i + 1)
        perm = list(range(lo, hi)) + [e for e in range(E) if not (lo <= e < hi)]
        in_maps.append(
            {
                "xT16": xT16,
                "gw": np.ascontiguousarray(gate_w[:, perm]).astype(np.float16),
                "gb": np.ascontiguousarray(gate_b[perm]).reshape(E, 1),
                "ones32": ones32,
                "w1": w1_16[lo:hi],
                "b1": np.ascontiguousarray(
                    b1[lo:hi].reshape(EL, HB, 128).transpose(2, 0, 1)
                ),
                "w2": w2_16[lo:hi],
                "b2T": b2[lo:hi].reshape(EL, CB, 128).astype(np.float16),
            }
        )
    return in_maps


def kernel(x, gate_w, gate_b, w1, b1, w2, b2, _trace=False, _tmpdir=None):
    nc = _get_nc()
    in_maps = make_in_maps(x, gate_w, gate_b, w1, b1, w2, b2)
    res = run_bass_kernel_spmd(
        nc,
        in_maps,
        core_ids=list(range(NCORES)),
        trace=_trace,
        tmpdir=_tmpdir,
    )
    acc = res.results[0]["outT"].astype(np.float64)
    for r in res.results[1:]:
        acc += r["outT"]
    out = acc.T.reshape(B, T, C).astype(np.float32)
    if _trace:
        kernel._last_results = res
    return out
